# revision 1
# baseline (speedup 1.0000x reference)
"""Trainium2 Bass kernel for an 8-expert top-2 SwiGLU MoE (expert parallelism).

Structure (8 NeuronCores, one expert per core):
  - Stage 1 (gating): stream the full transposed token set xT (f32), compute
    logits for all 8192 tokens on the PE in f32r (full-rate), with the gate
    matrix column-permuted per core so its OWN expert is column 0. Top-2 via
    vector MAX8 per 128-token block; routing weights + mask via batched DVE
    ops per 1024-token chunk; compact slot ids via matmul prefix-sums;
    per-block indirect scatters write (token_id, weight_bits) into idw_d.
  - Stage 2 (FFN): bf16 weights and activations (weights marshaled to
    contiguous-packed bf16 on host). Slot range processed in 3 chunks
    (1024/1024/256); per chunk: indirect-gather token rows (f32), PE
    transpose to feature-major with cast to bf16, pass1
    (silu(x@w1T)*(x@w3T)), pass2 (h@w2T) written feature-major unscaled to
    yT_d[D, C] in f32.
  - Host: y[ids] += w[:, None] * yT[:, :cnt].T per core.

Self-contained: hardcodes shapes for x[4,2048,1024], 8 experts, H=2816, top-2.
"""
import sys

sys.path.insert(0, "/opt/trn_rl_repo")

import numpy as np
from ml_dtypes import bfloat16

# ---------------------------------------------------------------- config
B, S, D = 4, 2048, 1024
T = B * S                # 8192 tokens
E = 8                    # experts == cores
H = 2816
K = 2
P = 128
NB = T // P              # 64 token blocks (token = 128*b + p)
C = 2304                 # per-expert slot capacity (observed max 2175)
NG = C // P              # 18 slot tiles
HT = H // P              # 22
DT = D // P              # 8
GATE_CHUNK = 1024
NJ = T // GATE_CHUNK     # 8
BPC = GATE_CHUNK // P    # 8 blocks per gating chunk
CHUNKS = [(0, 1024), (1024, 1024), (2048, 256)]
SLICE = 512

_cache = {}


def _build():
    import concourse.bass as bass
    import concourse.bacc as bacc
    import concourse.mybir as mybir
    import concourse.tile as tile

    f32 = mybir.dt.float32
    f32r = mybir.dt.float32r
    bf16 = mybir.dt.bfloat16
    i32 = mybir.dt.int32
    Alu = mybir.AluOpType
    Act = mybir.ActivationFunctionType

    nc = bacc.Bacc("TRN2", target_bir_lowering=False, debug=False)

    x_d = nc.dram_tensor("x", [T, D], f32, kind="ExternalInput")
    xT_d = nc.dram_tensor("xT", [D, T], f32, kind="ExternalInput")
    gwP_d = nc.dram_tensor("gwP", [P, DT * E], f32, kind="ExternalInput")
    w1T_d = nc.dram_tensor("w1T", [D, H], f32r, kind="ExternalInput")
    w3T_d = nc.dram_tensor("w3T", [D, H], f32r, kind="ExternalInput")
    w2T_d = nc.dram_tensor("w2T", [H, D], f32r, kind="ExternalInput")
    uexc_d = nc.dram_tensor("uexc", [P, P], f32, kind="ExternalInput")
    onesc_d = nc.dram_tensor("ones_col", [P, 1], f32, kind="ExternalInput")
    onesr_d = nc.dram_tensor("ones_row", [1, P], f32, kind="ExternalInput")
    iota_d = nc.dram_tensor("iota", [P, NB], i32, kind="ExternalInput")
    ident_d = nc.dram_tensor("ident", [P, P], f32, kind="ExternalInput")

    idw_d = nc.dram_tensor("idw", [C, 2], i32, kind="ExternalOutput")
    cnt_d = nc.dram_tensor("cnt", [1, 1], f32, kind="ExternalOutput")
    yT_d = nc.dram_tensor("yT", [D, C], f32, kind="ExternalOutput")

    with tile.TileContext(nc) as tc:
        with tc.tile_pool(name="persist", bufs=1) as sp:
            # --- constants ---
            uexc = sp.tile([P, P], f32)
            nc.sync.dma_start(out=uexc[:], in_=uexc_d[:])
            onesc = sp.tile([P, 1], f32)
            nc.sync.dma_start(out=onesc[:], in_=onesc_d[:])
            onesr = sp.tile([1, P], f32)
            nc.sync.dma_start(out=onesr[:], in_=onesr_d[:])
            iota = sp.tile([P, NB], i32)
            nc.sync.dma_start(out=iota[:], in_=iota_d[:])
            ident = sp.tile([P, P], f32)
            nc.sync.dma_start(out=ident[:], in_=ident_d[:])
            gws = sp.tile([P, DT * E], f32)
            nc.sync.dma_start(out=gws[:], in_=gwP_d[:])

            # PE wait-absorber: matmul codegen allows a single sync wait, so
            # before any matmul that would need 2+ waits we make the PE observe
            # the extra semaphores through a tiny dummy matmul.
            dummy_ps = None

            def pe_touch(ap):
                n = ap.shape[-1]
                nc.tensor.matmul(dummy_ps[0:1, 0:n], lhsT=ap[:, 0:1], rhs=ap,
                                 start=True, stop=True, skip_group_check=True)

            scores = sp.tile([P, NB * E], f32)     # [p, b*E+e] logits (perm'd)
            mx_all = sp.tile([P, NB * 8], f32)     # per-block top-8 descending
            incl_all = sp.tile([1, NB], f32)

            # ---------------- stage 1: gating + routing ----------------
            with tc.tile_pool(name="gpsum", bufs=2, space="PSUM") as ppg, \
                 tc.tile_pool(name="gsb", bufs=3) as sg:
                dummy_ps = ppg.tile([1, 2], f32, tag="dummy", bufs=1)
                pe_touch(gws[0:1, 0:2])
                pe_touch(ident[0:1, 0:2])
                pe_touch(uexc[0:1, 0:2])
                pe_touch(onesc[0:1, 0:1])
                pe_touch(onesr[0:1, 0:2])
                sc3 = scores[:].rearrange("p (b e) -> p b e", e=E)
                mx3 = mx_all[:].rearrange("p (b e) -> p b e", e=8)
                for j in range(NJ):
                    b0 = j * BPC
                    xt = sg.tile([P, DT, GATE_CHUNK], f32, tag="xt", bufs=2)
                    nc.sync.dma_start(
                        out=xt[:],
                        in_=xT_d[:].rearrange("(k p) t -> p k t", p=P)[:, :, j * GATE_CHUNK:(j + 1) * GATE_CHUNK])
                    ps = ppg.tile([E, GATE_CHUNK], f32, tag="ps", space="PSUM")
                    for h0 in range(0, GATE_CHUNK, 512):
                        for k in range(DT):
                            nc.tensor.matmul(ps[:, h0:h0 + 512],
                                             lhsT=gws[:, k * E:(k + 1) * E],
                                             rhs=xt[:, k, h0:h0 + 512],
                                             start=(k == 0), stop=(k == DT - 1))
                    sc_sb = sg.tile([E, GATE_CHUNK], f32, tag="sc")
                    nc.vector.tensor_copy(out=sc_sb[:], in_=ps[:])
                    pstb = ppg.tile([P, BPC * E], f32, tag="pst", space="PSUM")
                    for i in range(BPC):
                        nc.tensor.transpose(out=pstb[:, i * E:(i + 1) * E],
                                            in_=sc_sb[:, i * P:(i + 1) * P],
                                            identity=ident[0:E, 0:E])
                    nc.vector.tensor_copy(out=scores[:, b0 * E:(b0 + BPC) * E],
                                          in_=pstb[:])
                    for i in range(BPC):
                        nc.vector.max(out=mx_all[:, (b0 + i) * 8:(b0 + i + 1) * 8],
                                      in_=scores[:, (b0 + i) * E:(b0 + i + 1) * E])

                    m1j = mx3[:, b0:b0 + BPC, 0]
                    m2j = mx3[:, b0:b0 + BPC, 1]
                    sej = sc3[:, b0:b0 + BPC, 0]     # own expert is column 0
                    dlt = sg.tile([P, BPC], f32, tag="dlt")
                    nc.vector.tensor_sub(out=dlt[:], in0=m2j, in1=m1j)
                    ed = sg.tile([P, BPC], f32, tag="ed")
                    nc.scalar.activation(out=ed[:], in_=dlt[:], func=Act.Exp)
                    den = sg.tile([P, BPC], f32, tag="den")
                    nc.vector.tensor_scalar_add(den[:], ed[:], 1.0)
                    wtop = sg.tile([P, BPC], f32, tag="wtop")
                    nc.vector.reciprocal(out=wtop[:], in_=den[:])
                    wsec = sg.tile([P, BPC], f32, tag="wsec")
                    nc.vector.tensor_scalar(out=wsec[:], in0=wtop[:], scalar1=-1.0,
                                            scalar2=1.0, op0=Alu.mult, op1=Alu.add)
                    istop = sg.tile([P, BPC], f32, tag="istop")
                    nc.vector.tensor_tensor(out=istop[:], in0=sej, in1=m1j, op=Alu.is_ge)
                    wdiff = sg.tile([P, BPC], f32, tag="wdiff")
                    nc.vector.tensor_sub(out=wdiff[:], in0=wtop[:], in1=wsec[:])
                    wE = sg.tile([P, BPC], f32, tag="wE")
                    nc.vector.tensor_tensor(out=wE[:], in0=istop[:], in1=wdiff[:], op=Alu.mult)
                    nc.vector.tensor_add(out=wE[:], in0=wE[:], in1=wsec[:])
                    maskj = sg.tile([P, BPC], f32, tag="maskj")
                    nc.vector.tensor_tensor(out=maskj[:], in0=sej, in1=m2j, op=Alu.is_ge)

                    pslot = ppg.tile([P, BPC], f32, tag="pslot", space="PSUM", bufs=1)
                    nc.tensor.matmul(pslot[:], lhsT=uexc[:], rhs=maskj[:], start=True, stop=False)
                    ptot = ppg.tile([1, BPC], f32, tag="dummy", space="PSUM", bufs=1)
                    nc.tensor.matmul(ptot[:], lhsT=onesc[:], rhs=maskj[:], start=True, stop=True)
                    tot = sg.tile([1, BPC], f32, tag="tot")
                    nc.vector.tensor_copy(out=tot[:], in_=ptot[:])
                    init = 0.0 if j == 0 else incl_all[:, b0 - 1:b0]
                    nc.vector.tensor_tensor_scan(incl_all[:, b0:b0 + BPC], tot[:], tot[:], init,
                                                 op0=Alu.add, op1=Alu.bypass)
                    excl = sg.tile([1, BPC], f32, tag="excl")
                    nc.vector.tensor_sub(out=excl[:], in0=incl_all[:, b0:b0 + BPC], in1=tot[:])
                    nc.tensor.matmul(pslot[:], lhsT=onesr[:], rhs=excl[:], start=False, stop=True)
                    slot_f = sg.tile([P, BPC], f32, tag="slot_f")
                    nc.vector.tensor_copy(out=slot_f[:], in_=pslot[:])
                    off_f = sg.tile([P, BPC], f32, tag="off_f")
                    nc.vector.tensor_scalar(out=off_f[:], in0=maskj[:], scalar1=-1e6,
                                            scalar2=1e6, op0=Alu.mult, op1=Alu.add)
                    slot_oob = sg.tile([P, BPC], f32, tag="slot_oob")
                    nc.vector.tensor_add(out=slot_oob[:], in0=slot_f[:], in1=off_f[:])
                    slot_i = sg.tile([P, BPC], i32, tag="slot_i")
                    nc.vector.tensor_copy(out=slot_i[:], in_=slot_oob[:])
                    iw = sg.tile([P, 2 * BPC], i32, tag="iw")
                    iw3 = iw[:].rearrange("p (b two) -> p b two", two=2)
                    nc.vector.tensor_copy(out=iw3[:, :, 0], in_=iota[:, b0:b0 + BPC])
                    nc.vector.tensor_copy(out=iw3[:, :, 1], in_=wE[:].bitcast(i32))
                    for i in range(BPC):
                        nc.gpsimd.indirect_dma_start(
                            out=idw_d[:],
                            out_offset=bass.IndirectOffsetOnAxis(ap=slot_i[:, i:i + 1], axis=0),
                            in_=iw[:, 2 * i:2 * i + 2], in_offset=None,
                            bounds_check=C - 1, oob_is_err=False)

                cnt_sb = sg.tile([1, 1], f32, tag="cnt")
                nc.vector.tensor_copy(out=cnt_sb[:], in_=incl_all[:, NB - 1:NB])
                nc.sync.dma_start(out=cnt_d[:], in_=cnt_sb[:])

            # ---------------- stage 2: per super-chunk gather + FFN ----------------
            h_all = [sp.tile([P, 1024], f32r, tag=f"h{ht}", name=f"h{ht}") for ht in range(HT)]
            xgT = [sp.tile([P, 1024], f32r, tag=f"xgT{k}", name=f"xgT{k}") for k in range(DT)]
            idw_sb = [sp.tile([P, 2], i32, tag=f"idw{g}", name=f"idw{g}") for g in range(8)]

            for (c0, clen) in CHUNKS:
                ngc = clen // P
                nsl = (clen + SLICE - 1) // SLICE
                slices = [(s * SLICE, min(SLICE, clen - s * SLICE)) for s in range(nsl)]

                # gather + transpose to feature-major (cast to bf16)
                with tc.tile_pool(name="gat_ps", bufs=2, space="PSUM") as ppt, \
                     tc.tile_pool(name="gat_sb", bufs=3) as sgt:
                    dummy_ps = ppt.tile([1, 2], f32, tag="dummy", bufs=1)
                    for g in range(ngc):
                        gabs = c0 // P + g
                        nc.sync.dma_start(out=idw_sb[g][:], in_=idw_d[P * gabs:P * (gabs + 1), :])
                        xg = sgt.tile([P, D], f32, tag="xg")
                        nc.gpsimd.indirect_dma_start(
                            out=xg[:], out_offset=None, in_=x_d[:],
                            in_offset=bass.IndirectOffsetOnAxis(ap=idw_sb[g][:, 0:1], axis=0))
                        for k in range(DT):
                            pst = ppt.tile([P, P], f32, tag="pst", space="PSUM", bufs=4)
                            nc.tensor.transpose(out=pst[:], in_=xg[:, P * k:P * (k + 1)],
                                                identity=ident[:])
                            nc.vector.tensor_copy(out=xgT[k][:, g * P:(g + 1) * P], in_=pst[:])

                # FFN pass 1: h = silu(x@w1T) * (x@w3T)
                with tc.tile_pool(name="p1_ps", bufs=2, space="PSUM") as pp1, \
                     tc.tile_pool(name="p1_sb", bufs=3) as s1:
                    dummy_ps = pp1.tile([1, 2], f32, tag="dummy", bufs=1)
                    g_s0 = min(ngc, SLICE // P) - 1   # last slot-tile of slice 0
                    for k in range(DT):
                        pe_touch(xgT[k][0:1, g_s0 * P:g_s0 * P + 2])
                    prev_silu = None
                    for ht in range(HT):
                        w1b = s1.tile([P, DT, P], f32r, tag="w1b")
                        nc.sync.dma_start(
                            out=w1b[:],
                            in_=w1T_d[:].rearrange("(k p) h -> p k h", p=P)[:, :, ht * P:(ht + 1) * P])
                        w3b = s1.tile([P, DT, P], f32r, tag="w3b")
                        nc.sync.dma_start(
                            out=w3b[:],
                            in_=w3T_d[:].rearrange("(k p) h -> p k h", p=P)[:, :, ht * P:(ht + 1) * P])
                        for (s0, sl) in slices:
                            ph1 = pp1.tile([P, SLICE], f32, tag="ph1", space="PSUM")
                            ph3 = pp1.tile([P, SLICE], f32, tag="ph3", space="PSUM")
                            for k in range(DT):
                                nc.tensor.matmul(ph1[:, :sl], lhsT=w1b[:, k, :],
                                                 rhs=xgT[k][:, s0:s0 + sl],
                                                 start=(k == 0), stop=(k == DT - 1))
                            for k in range(DT):
                                nc.tensor.matmul(ph3[:, :sl], lhsT=w3b[:, k, :],
                                                 rhs=xgT[k][:, s0:s0 + sl],
                                                 start=(k == 0), stop=(k == DT - 1))
                            silu = s1.tile([P, SLICE], f32, tag="silu")
                            nc.scalar.activation(out=silu[:, :sl], in_=ph1[:, :sl], func=Act.Silu)
                            nc.vector.tensor_tensor(out=h_all[ht][:, s0:s0 + sl],
                                                    in0=silu[:, :sl], in1=ph3[:, :sl],
                                                    op=Alu.mult)
                            if prev_silu is not None:
                                pe_touch(prev_silu)
                            prev_silu = silu[0:1, 0:2]

                # FFN pass 2: yT = h @ w2T (feature-major, unscaled)
                with tc.tile_pool(name="p2_ps", bufs=2, space="PSUM") as pp2, \
                     tc.tile_pool(name="p2_sb", bufs=3) as s2:
                    dummy_ps = pp2.tile([1, 2], f32, tag="dummy", bufs=1)
                    for ht in range(HT):
                        pe_touch(h_all[ht][0:1, 0:2])
                    for dt in range(DT):
                        w2b = s2.tile([P, HT, P], f32r, tag="w2b")
                        nc.sync.dma_start(
                            out=w2b[:],
                            in_=w2T_d[:].rearrange("(k p) d -> p k d", p=P)[:, :, dt * P:(dt + 1) * P])
                        for (s0, sl) in slices:
                            py = pp2.tile([P, SLICE], f32, tag="py", space="PSUM")
                            for j in range(HT):
                                nc.tensor.matmul(py[:, :sl], lhsT=w2b[:, j, :],
                                                 rhs=h_all[j][:, s0:s0 + sl],
                                                 start=(j == 0), stop=(j == HT - 1))
                            ysb = s2.tile([P, SLICE], f32, tag="ysb")
                            nc.vector.tensor_copy(out=ysb[:, :sl], in_=py[:, :sl])
                            nc.sync.dma_start(
                                out=yT_d[dt * P:(dt + 1) * P, c0 + s0:c0 + s0 + sl],
                                in_=ysb[:, :sl])

    nc.compile()
    return nc


def _marshal(x, gate_w, w1, w3, w2):
    xf = np.ascontiguousarray(x.reshape(T, D).astype(np.float32))
    xT = np.ascontiguousarray(xf.T)
    consts = {
        "uexc": np.triu(np.ones((P, P), np.float32), 1),
        "ones_col": np.ones((P, 1), np.float32),
        "ones_row": np.ones((1, P), np.float32),
        "iota": (np.arange(P)[:, None] + P * np.arange(NB)[None, :]).astype(np.int32),
        "ident": np.eye(P, dtype=np.float32),
    }
    in_maps = []
    for e in range(E):
        perm = [e] + [i for i in range(E) if i != e]
        gwT = gate_w[perm].T.astype(np.float32)                      # [D, 8]
        gwP = np.ascontiguousarray(
            gwT.reshape(DT, P, E).transpose(1, 0, 2)).reshape(P, DT * E)
        w1T = np.ascontiguousarray(w1[e].astype(np.float32).T)
        w3T = np.ascontiguousarray(w3[e].astype(np.float32).T)
        w2T = np.ascontiguousarray(w2[e].astype(np.float32).T)
        in_maps.append({
            "x": xf, "xT": xT, "gwP": gwP,
            "w1T": w1T, "w3T": w3T, "w2T": w2T, **consts,
        })
    return in_maps


def _numpy_fallback(x, gate_w, w1, w3, w2):
    xf = x.reshape(T, D).astype(np.float64)
    logits = xf @ gate_w.astype(np.float64).T
    p = np.exp(logits - logits.max(1, keepdims=True))
    p /= p.sum(1, keepdims=True)
    idx = np.argsort(-p, axis=1, kind="stable")[:, :K]
    vals = np.take_along_axis(p, idx, 1)
    vals /= vals.sum(1, keepdims=True)
    y = np.zeros_like(xf)
    for e in range(E):
        m = (idx == e)
        wgt = (vals * m).sum(1)
        tsel = m.any(1)
        xe = xf[tsel]
        hm = xe @ w1[e].astype(np.float64).T
        hm = hm / (1 + np.exp(-hm)) * (xe @ w3[e].astype(np.float64).T)
        y[tsel] += wgt[tsel, None] * (hm @ w2[e].astype(np.float64).T)
    return y.astype(np.float32).reshape(x.shape)


def run_spmd(x, gate_w, w1, w3, w2, trace=False):
    """Compile (cached), run on 8 cores, return results."""
    from concourse.bass_utils import run_bass_kernel_spmd
    if "nc" not in _cache:
        _cache["nc"] = _build()
    in_maps = _marshal(x, gate_w, w1, w3, w2)
    res = run_bass_kernel_spmd(_cache["nc"], in_maps, list(range(E)), trace=trace)
    return res


def kernel(x, gate_w, w1, w3, w2):
    x = np.asarray(x)
    res = run_spmd(x, gate_w, w1, w3, w2)
    y = np.zeros((T, D), np.float32)
    for e in range(E):
        r = res.results[e]
        cnt = int(round(float(r["cnt"][0, 0])))
        if cnt > C:
            return _numpy_fallback(x, gate_w, w1, w3, w2)
        ids = r["idw"][:cnt, 0]
        w = r["idw"][:cnt, 1].view(np.float32)
        rows = r["yT"][:, :cnt].T
        if len(np.unique(ids)) == cnt:
            y[ids] += w[:, None] * rows
        else:
            np.add.at(y, ids, w[:, None] * rows)
    return y.reshape(x.shape)

